# revision 7
# baseline (speedup 1.0000x reference)
"""BERT encoder forward pass on 8 TRN2 NeuronCores.

Strategy: pure data parallelism over the batch (16 sequences -> 2 per core).
Each core runs the full 12-layer encoder on its 2 sequences; no collectives.

Device kernel layout conventions (per core, T = 2*512 = 1024 tokens):
  h    : residual stream, fp32, non-transposed   [t(8x128 part), d(768 free)]
  hT   : bf16 transposed copy of h               [d(6x128 part), t(1024 free)]
  qaug : per (seq,head) "augmented" q^T, bf16    [128 part = 64 dk + ones row
         + zero pad, 512 free = query pos]; the ones row multiplies the mask
         row of kaug inside the score matmul, so masking is free.
  kaug : same layout, row 64 = additive key mask (0 / -1e9)
  v    : bf16 non-transposed with a ones column per head  [t part, 12*(64+1)]
         -> the ones column makes the ctx matmul also emit the softmax
         denominator as psum row 64.
All matmuls contract over the partition dim with fp32 PSUM accumulation.
Softmax skips max-subtraction (scores are O(1); masked logits underflow to 0).
"""

import os

import ml_dtypes
import numpy as np

B, L, D, NL, H, DK, FF = 16, 512, 768, 12, 12, 64, 3072
NCORES = 8
SPC = B // NCORES          # sequences per core
T = SPC * L                # tokens per core
DCH = D // 128             # 6 chunks of d
FCH = FF // 128            # 24 chunks of ff
TCH = T // 128             # 8 chunks of t
LCH = L // 128             # 4 chunks of one sequence
DH = D // 2                # 384: n-split of 768-wide matmul outputs

_CACHE = {}


def _build_program(nl, use_bias, use_affine):
    import concourse.mybir as mybir
    import concourse.tile as tile
    from concourse import bacc
    from concourse.masks import make_identity

    f32, bf16 = mybir.dt.float32, mybir.dt.bfloat16
    AF = mybir.ActivationFunctionType
    OP = mybir.AluOpType

    nc = bacc.Bacc("TRN2", target_bir_lowering=False, debug=False)
    h0_d = nc.dram_tensor("h0", [T, D], f32, kind="ExternalInput").ap()
    mask_d = nc.dram_tensor("maskrow", [SPC, L], bf16, kind="ExternalInput").ap()
    wq_d = nc.dram_tensor("wq", [nl, D, D], bf16, kind="ExternalInput").ap()
    wk_d = nc.dram_tensor("wk", [nl, D, D], bf16, kind="ExternalInput").ap()
    wv_d = nc.dram_tensor("wv", [nl, D, D], bf16, kind="ExternalInput").ap()
    wo_d = nc.dram_tensor("wo", [nl, D, D], bf16, kind="ExternalInput").ap()
    w1_d = nc.dram_tensor("w1", [nl, D, FF], bf16, kind="ExternalInput").ap()
    w2_d = nc.dram_tensor("w2", [nl, FF, D], bf16, kind="ExternalInput").ap()
    names = ["h0", "maskrow", "wq", "wk", "wv", "wo", "w1", "w2"]
    if use_bias:
        bqk_d = nc.dram_tensor("bqk", [nl, 2, D], f32, kind="ExternalInput").ap()
        b1_d = nc.dram_tensor("b1", [nl, FF], f32, kind="ExternalInput").ap()
        brow_d = nc.dram_tensor("brow", [nl, 3, D], bf16, kind="ExternalInput").ap()
        names += ["bqk", "b1", "brow"]
    if use_affine:
        lng_d = nc.dram_tensor("lng", [nl, D], f32, kind="ExternalInput").ap()
        lnb_d = nc.dram_tensor("lnb", [nl, D], f32, kind="ExternalInput").ap()
        names += ["lng", "lnb"]
    out_d = nc.dram_tensor("out", [T, D], f32, kind="ExternalOutput").ap()

    with tile.TileContext(nc) as tc:
        with (
            tc.tile_pool(name="const", bufs=1) as cp,
            tc.tile_pool(name="persist", bufs=1) as pp,
            tc.tile_pool(name="wts", bufs=1) as wp,
            tc.tile_pool(name="work", bufs=2) as wk,
            tc.tile_pool(name="psum", bufs=2, space="PSUM") as psp,
        ):
            ident = cp.tile([128, 128], f32)
            make_identity(nc, ident)
            identb = cp.tile([128, 128], bf16)
            nc.vector.tensor_copy(identb[:], ident[:])
            eps_t = cp.tile([128, 1], f32)
            nc.vector.memset(eps_t[:], 1e-5)
            if use_bias:
                ones_row = cp.tile([1, 128], bf16)
                nc.vector.memset(ones_row[:], 1.0)

            h_t = [pp.tile([128, D], f32, tag=f"h{i}", name=f"h{i}") for i in range(TCH)]
            hT_t = [pp.tile([128, T], bf16, tag=f"hT{c}", name=f"hT{c}") for c in range(DCH)]
            qaug = [[pp.tile([128, L], bf16, tag=f"qa{s}_{hd}", name=f"qa{s}_{hd}") for hd in range(H)]
                    for s in range(SPC)]
            kaug = [[pp.tile([128, L], bf16, tag=f"ka{s}_{hd}", name=f"ka{s}_{hd}") for hd in range(H)]
                    for s in range(SPC)]
            v_t = [pp.tile([128, H, DK + 1], bf16, tag=f"v{i}", name=f"v{i}") for i in range(TCH)]
            # ctxT aliases the hT tiles: within a layer, all hT reads (qk/v
            # matmuls) complete before attention writes ctx into the same
            # slots; Tile's WAR tracking enforces the ordering.
            ctxT = hT_t
            gel = [pp.tile([128, T], bf16, tag=f"g{f}", name=f"g{f}") for f in range(FCH)]

            # one-time init: zero the aug tiles, set ones/mask rows + v ones col
            for s in range(SPC):
                for hd in range(H):
                    nc.vector.memset(qaug[s][hd][:], 0.0)
                    nc.vector.memset(kaug[s][hd][:], 0.0)
                    nc.vector.memset(qaug[s][hd][DK:DK + 1, :], 1.0)
                    nc.sync.dma_start(kaug[s][hd][DK:DK + 1, :], mask_d[s:s + 1, :])
            for i in range(TCH):
                nc.vector.memset(v_t[i][:, :, DK:DK + 1], 1.0)
                nc.sync.dma_start(h_t[i][:], h0_d[i * 128:(i + 1) * 128, :])

            def transpose_h():
                # two rounds of 4 token-tiles to halve h16 SBUF footprint
                for r in range(2):
                    h16 = [wk.tile([128, D], bf16, tag=f"h16_{j}", bufs=1,
                                   name=f"h16_{j}") for j in range(4)]
                    for j in range(4):
                        nc.scalar.activation(h16[j][:], h_t[4 * r + j][:], AF.Copy)
                    for c in range(DCH):
                        pst = psp.tile([128, 512], bf16, tag="pstb", bufs=2)
                        for j in range(4):
                            nc.tensor.transpose(
                                pst[:, j * 128:(j + 1) * 128],
                                h16[j][:, c * 128:(c + 1) * 128], identb[:])
                        nc.vector.tensor_copy(hT_t[c][:, r * 512:(r + 1) * 512],
                                              pst[:])

            def layernorm(gb):
                for i in range(TCH):
                    st = wk.tile([128, 2, 6], f32, tag="bnst", bufs=1)
                    for g in range(2):
                        nc.vector.bn_stats(st[:, g, :], h_t[i][:, g * 384:(g + 1) * 384])
                    mv = wk.tile([128, 2], f32, tag="bnmv", bufs=1)
                    nc.vector.bn_aggr(mv[:], st[:])
                    rstd = wk.tile([128, 1], f32, tag="rstd", bufs=1)
                    nc.scalar.activation(rstd[:], mv[:, 1:2], AF.Sqrt, bias=eps_t[:])
                    nc.vector.reciprocal_approx_fast(rstd[:], rstd[:])
                    nc.vector.tensor_scalar(h_t[i][:], h_t[i][:], mv[:, 0:1], rstd[:],
                                            op0=OP.subtract, op1=OP.mult)
                    if gb is not None:
                        nc.vector.tensor_tensor(h_t[i][:], h_t[i][:], gb[0][:],
                                                op=OP.mult)
                        nc.vector.tensor_tensor(h_t[i][:], h_t[i][:], gb[1][:],
                                                op=OP.add)

            transpose_h()

            for l in range(nl):
                if use_bias:
                    bqk_sb = wk.tile([128, 2 * DCH], f32, tag="bqk")
                    nc.sync.dma_start(bqk_sb[:],
                                      bqk_d[l].rearrange("b (c p) -> p (b c)", p=128))
                    b1_sb = wk.tile([128, FCH], f32, tag="b1sb")
                    nc.sync.dma_start(b1_sb[:],
                                      b1_d[l].rearrange("(c p) -> p c", p=128))
                    brow_sb = wk.tile([3, D], bf16, tag="brow")
                    nc.sync.dma_start(brow_sb[:], brow_d[l])
                if use_affine:
                    g_bc = wk.tile([128, D], f32, tag="gbc")
                    b_bc = wk.tile([128, D], f32, tag="bbc")
                    nc.gpsimd.dma_start(g_bc[:], lng_d[l:l + 1, :].to_broadcast((128, D)))
                    nc.gpsimd.dma_start(b_bc[:], lnb_d[l:l + 1, :].to_broadcast((128, D)))
                    gb = (g_bc, b_bc)
                else:
                    gb = None

                # ---- q^T / k^T (into aug tiles, scale q by 1/sqrt(dk)) ----
                for mat_d, aug, scale, bcol in ((wq_d, qaug, 1.0, 0),
                                                (wk_d, kaug, 1.0, 1)):
                    for dc in range(DCH):
                        wc = wk.tile([128, DCH, 128], bf16, tag="wqkcol", bufs=2, name="wqkc")
                        nc.sync.dma_start(
                            wc[:], mat_d[l][:, dc * 128:(dc + 1) * 128]
                            .rearrange("(c p) n -> p c n", p=128))
                        ps = psp.tile([128, T], f32, tag="ps2", bufs=2)
                        for c in range(DCH):
                            for s in range(SPC):
                                nc.tensor.matmul(ps[:, s * L:(s + 1) * L], wc[:, c, :],
                                                 hT_t[c][:, s * L:(s + 1) * L],
                                                 start=(c == 0), stop=(c == DCH - 1))
                        for s in range(SPC):
                            for hh in range(2):
                                src = ps[hh * 64:(hh + 1) * 64, s * L:(s + 1) * L]
                                dst = aug[s][2 * dc + hh][0:DK, :]
                                if use_bias:
                                    nc.scalar.activation(
                                        dst, src, AF.Identity, scale=scale,
                                        bias=bqk_sb[hh * 64:(hh + 1) * 64,
                                                    bcol * DCH + dc:bcol * DCH + dc + 1])
                                else:
                                    nc.vector.tensor_copy(dst, src)

                # ---- v (non-transposed, interleaved with ones cols) ----
                wv_rows = []
                for c in range(DCH):
                    wr = wp.tile([128, D], bf16, tag=f"wv{c}", name=f"wv{c}")
                    nc.sync.dma_start(wr[:], wv_d[l, c * 128:(c + 1) * 128, :])
                    wv_rows.append(wr)
                for i in range(TCH):
                    ps = psp.tile([128, D], f32, tag="ps2", bufs=2)
                    for c in range(DCH):
                        for off, wdt in ((0, 512), (512, 256)):
                            nc.tensor.matmul(ps[:, off:off + wdt],
                                             hT_t[c][:, i * 128:(i + 1) * 128],
                                             wv_rows[c][:, off:off + wdt],
                                             start=(c == 0),
                                             stop=(c == DCH - 1 and not use_bias))
                    if use_bias:
                        for off, wdt in ((0, 512), (512, 256)):
                            nc.tensor.matmul(ps[:, off:off + wdt], ones_row[:],
                                             brow_sb[0:1, off:off + wdt],
                                             start=False, stop=True)
                    nc.vector.tensor_copy(v_t[i][:, :, 0:DK],
                                          ps[:].rearrange("p (h e) -> p h e", e=DK))

                # ---- attention per (sequence, head) ----
                for s in range(SPC):
                    for hd in range(H):
                        nm = []
                        for tp_ in range(2):
                            ps = psp.tile([128, T], f32, tag="ps2", bufs=2)
                            for half in range(2):
                                tk = 2 * tp_ + half
                                nc.tensor.matmul(ps[:, half * L:(half + 1) * L],
                                                 kaug[s][hd][:, tk * 128:(tk + 1) * 128],
                                                 qaug[s][hd][:], start=True, stop=True)
                            t_nm = wk.tile([128, T], bf16, tag="numer", bufs=3,
                                           name="nm")
                            nc.scalar.activation(t_nm[:], ps[:], AF.Exp)
                            nm.append(t_nm)
                        cps = psp.tile([128, L], f32, tag="ps1", bufs=2)
                        for tk in range(LCH):
                            nc.tensor.matmul(cps[0:DK + 1, :],
                                             v_t[s * LCH + tk][:, hd, :],
                                             nm[tk // 2][:, (tk % 2) * L:(tk % 2 + 1) * L],
                                             start=(tk == 0), stop=(tk == LCH - 1))
                        rec = wk.tile([1, L], f32, tag="rec", bufs=2)
                        nc.vector.tensor_copy(rec[:], cps[DK:DK + 1, :])
                        nc.vector.reciprocal_approx_fast(rec[:], rec[:])
                        recb = wk.tile([DK, L], f32, tag="recb", bufs=2)
                        nc.gpsimd.partition_broadcast(recb[:], rec[:])
                        dst = ctxT[hd // 2][(hd % 2) * 64:(hd % 2) * 64 + 64,
                                            s * L:(s + 1) * L]
                        nc.vector.tensor_tensor(dst, cps[0:DK, :], recb[:],
                                                op=OP.mult)

                # ---- attn output + residual ----
                wo_rows = []
                for c in range(DCH):
                    wr = wp.tile([128, D], bf16, tag=f"wo{c}", name=f"wo{c}")
                    nc.sync.dma_start(wr[:], wo_d[l, c * 128:(c + 1) * 128, :])
                    wo_rows.append(wr)
                for i in range(TCH):
                    ps = psp.tile([128, D], f32, tag="ps2", bufs=2)
                    for c in range(DCH):
                        for off, wdt in ((0, 512), (512, 256)):
                            nc.tensor.matmul(ps[:, off:off + wdt],
                                             ctxT[c][:, i * 128:(i + 1) * 128],
                                             wo_rows[c][:, off:off + wdt],
                                             start=(c == 0),
                                             stop=(c == DCH - 1 and not use_bias))
                    if use_bias:
                        for off, wdt in ((0, 512), (512, 256)):
                            nc.tensor.matmul(ps[:, off:off + wdt], ones_row[:],
                                             brow_sb[1:2, off:off + wdt],
                                             start=False, stop=True)
                    nc.vector.tensor_tensor(h_t[i][:], ps[:], h_t[i][:], op=OP.add)

                layernorm(gb)
                transpose_h()

                # ---- ffn1 + gelu (transposed output) ----
                for f in range(FCH):
                    w1c = wk.tile([128, DCH, 128], bf16, tag="w1col", bufs=2, name="w1c")
                    nc.sync.dma_start(
                        w1c[:], w1_d[l][:, f * 128:(f + 1) * 128]
                        .rearrange("(c p) n -> p c n", p=128))
                    ps = psp.tile([128, T], f32, tag="ps2", bufs=2)
                    for c in range(DCH):
                        for g in range(2):
                            nc.tensor.matmul(ps[:, g * 512:(g + 1) * 512], w1c[:, c, :],
                                             hT_t[c][:, g * 512:(g + 1) * 512],
                                             start=(c == 0), stop=(c == DCH - 1))
                    nc.scalar.activation(
                        gel[f][:], ps[:], AF.Gelu,
                        bias=(b1_sb[:, f:f + 1] if use_bias else 0.0))

                # ---- ffn2 + residual ----
                for dh in range(2):
                    w2_rows = []
                    for f in range(FCH):
                        wr = wp.tile([128, DH], bf16, tag=f"w2r{f}", name=f"w2r{f}")
                        nc.sync.dma_start(
                            wr[:], w2_d[l, f * 128:(f + 1) * 128,
                                        dh * DH:(dh + 1) * DH])
                        w2_rows.append(wr)
                    for i in range(TCH):
                        ps = psp.tile([128, DH], f32, tag="ps1", bufs=2)
                        for f in range(FCH):
                            nc.tensor.matmul(ps[:], gel[f][:, i * 128:(i + 1) * 128],
                                             w2_rows[f][:],
                                             start=(f == 0),
                                             stop=(f == FCH - 1 and not use_bias))
                        if use_bias:
                            nc.tensor.matmul(ps[:], ones_row[:],
                                             brow_sb[2:3, dh * DH:(dh + 1) * DH],
                                             start=False, stop=True)
                        nc.vector.tensor_tensor(h_t[i][:, dh * DH:(dh + 1) * DH], ps[:],
                                                h_t[i][:, dh * DH:(dh + 1) * DH],
                                                op=OP.add)

                layernorm(gb)
                if l < nl - 1:
                    transpose_h()

            for i in range(TCH):
                nc.sync.dma_start(out_d[i * 128:(i + 1) * 128, :], h_t[i][:])

    nc.compile()
    return nc, names


def _get_program(nl, use_bias, use_affine):
    key = (nl, use_bias, use_affine)
    if key not in _CACHE:
        _CACHE[key] = _build_program(nl, use_bias, use_affine)
    return _CACHE[key]


def kernel(**inputs):
    from concourse import bass_utils

    x = np.asarray(inputs["x"])
    tok = np.asarray(inputs["token_emb"], np.float32)
    pe = np.asarray(inputs["pe"], np.float32)
    to_bf = lambda a: np.asarray(a, np.float32).astype(ml_dtypes.bfloat16)

    h0 = tok[x] + pe[None]                                   # (B, L, D) f32
    maskrow = np.where(x > 0, 0.0, -1e9).astype(np.float32)  # (B, L)

    nl = int(os.environ.get("BERT_NL", str(NL)))
    bias_arrs = [np.asarray(inputs[k], np.float32)[:nl]
                 for k in ("bq", "bk", "bv", "bo", "b1", "b2")]
    use_bias = any(np.any(a != 0.0) for a in bias_arrs)
    lng = np.asarray(inputs["ln_g"], np.float32)[:nl]
    lnb = np.asarray(inputs["ln_b"], np.float32)[:nl]
    use_affine = bool(np.any(lng != 1.0) or np.any(lnb != 0.0))

    nc, names = _get_program(nl, use_bias, use_affine)

    shared = {
        "wq": to_bf(np.asarray(inputs["Wq"][:nl], np.float32) * 0.125),
        "wk": to_bf(inputs["Wk"][:nl]),
        "wv": to_bf(inputs["Wv"][:nl]), "wo": to_bf(inputs["Wo"][:nl]),
        "w1": to_bf(inputs["W1"][:nl]), "w2": to_bf(inputs["W2"][:nl]),
    }
    if use_bias:
        bq, bk, bv, bo, b1, b2 = bias_arrs
        shared["bqk"] = np.stack([bq, bk], axis=1).astype(np.float32)
        shared["b1"] = b1.astype(np.float32)
        shared["brow"] = to_bf(np.stack([bv, bo, b2], axis=1))
    if use_affine:
        shared["lng"] = lng
        shared["lnb"] = lnb

    in_maps = []
    for c in range(NCORES):
        im = dict(shared)
        im["h0"] = np.ascontiguousarray(
            h0[SPC * c:SPC * (c + 1)].reshape(T, D), dtype=np.float32)
        im["maskrow"] = to_bf(maskrow[SPC * c:SPC * (c + 1)])
        in_maps.append(im)

    trace = os.environ.get("BERT_TRACE", "0") == "1"
    res = bass_utils.run_bass_kernel_spmd(
        nc, in_maps, core_ids=list(range(NCORES)), trace=trace)
    if trace:
        print(f"HW exec time: {res.exec_time_ns} ns")

    out = np.stack([np.asarray(res.results[c]["out"]).reshape(SPC, L, D)
                    for c in range(NCORES)])
    return out.reshape(B, L, D).astype(np.float32)



# revision 9
# speedup vs baseline: 1.0610x; 1.0610x over previous
"""BERT encoder forward pass on 8 TRN2 NeuronCores.

Strategy: pure data parallelism over the batch (16 sequences -> 2 per core).
Each core runs the full 12-layer encoder on its 2 sequences; no collectives.

Device kernel layout conventions (per core, T = 2*512 = 1024 tokens):
  h    : residual stream, fp32, non-transposed   [t(8x128 part), d(768 free)]
  hT   : bf16 transposed copy of h               [d(6x128 part), t(1024 free)]
  qaug : per (seq,head) "augmented" q^T, bf16    [128 part = 64 dk + ones row
         + zero pad, 512 free = query pos]; the ones row multiplies the mask
         row of kaug inside the score matmul, so masking is free.
  kaug : same layout, row 64 = additive key mask (0 / -1e9)
  v    : bf16 non-transposed with a ones column per head  [t part, 12*(64+1)]
         -> the ones column makes the ctx matmul also emit the softmax
         denominator as psum row 64.
All matmuls contract over the partition dim with fp32 PSUM accumulation.
Softmax skips max-subtraction (scores are O(1); masked logits underflow to 0).
"""

import os

import ml_dtypes
import numpy as np

B, L, D, NL, H, DK, FF = 16, 512, 768, 12, 12, 64, 3072
NCORES = 8
SPC = B // NCORES          # sequences per core
T = SPC * L                # tokens per core
DCH = D // 128             # 6 chunks of d
FCH = FF // 128            # 24 chunks of ff
TCH = T // 128             # 8 chunks of t
LCH = L // 128             # 4 chunks of one sequence
DH = D // 2                # 384: n-split of 768-wide matmul outputs

_CACHE = {}


def _build_program(nl, use_bias, use_affine):
    import concourse.mybir as mybir
    import concourse.tile as tile
    from concourse import bacc
    from concourse.masks import make_identity

    f32, bf16 = mybir.dt.float32, mybir.dt.bfloat16
    e4, e5 = mybir.dt.float8e4, mybir.dt.float8e5
    DR = mybir.MatmulPerfMode.DoubleRow
    AF = mybir.ActivationFunctionType
    OP = mybir.AluOpType

    nc = bacc.Bacc("TRN2", target_bir_lowering=False, debug=False)
    h0_d = nc.dram_tensor("h0", [T, D], f32, kind="ExternalInput").ap()
    mask_d = nc.dram_tensor("maskrow", [SPC, L], bf16, kind="ExternalInput").ap()
    wq_d = nc.dram_tensor("wq", [nl, D, D], bf16, kind="ExternalInput").ap()
    wk_d = nc.dram_tensor("wk", [nl, D, D], bf16, kind="ExternalInput").ap()
    wv_d = nc.dram_tensor("wv", [nl, D, D], bf16, kind="ExternalInput").ap()
    wo_d = nc.dram_tensor("wo", [nl, D, D], bf16, kind="ExternalInput").ap()
    w1_d = nc.dram_tensor("w1", [nl, D, FF], bf16, kind="ExternalInput").ap()
    w2_d = nc.dram_tensor("w2", [nl, FF, D], bf16, kind="ExternalInput").ap()
    names = ["h0", "maskrow", "wq", "wk", "wv", "wo", "w1", "w2"]
    if use_bias:
        bqk_d = nc.dram_tensor("bqk", [nl, 2, D], f32, kind="ExternalInput").ap()
        b1_d = nc.dram_tensor("b1", [nl, FF], f32, kind="ExternalInput").ap()
        brow_d = nc.dram_tensor("brow", [nl, 3, D], bf16, kind="ExternalInput").ap()
        names += ["bqk", "b1", "brow"]
    if use_affine:
        lng_d = nc.dram_tensor("lng", [nl, D], f32, kind="ExternalInput").ap()
        lnb_d = nc.dram_tensor("lnb", [nl, D], f32, kind="ExternalInput").ap()
        names += ["lng", "lnb"]
    out_d = nc.dram_tensor("out", [T, D], f32, kind="ExternalOutput").ap()

    with tile.TileContext(nc) as tc:
        with (
            tc.tile_pool(name="const", bufs=1) as cp,
            tc.tile_pool(name="persist", bufs=1) as pp,
            tc.tile_pool(name="wts", bufs=1) as wp,
            tc.tile_pool(name="work", bufs=2) as wk,
            tc.tile_pool(name="psum", bufs=2, space="PSUM") as psp,
        ):
            ident = cp.tile([128, 128], f32)
            make_identity(nc, ident)
            eps_t = cp.tile([128, 1], f32)
            nc.vector.memset(eps_t[:], 1e-5)
            shift_t = cp.tile([128, 1], f32)
            nc.vector.memset(shift_t[:], -8.0)
            if use_bias:
                ones_row = cp.tile([1, 128], bf16)
                nc.vector.memset(ones_row[:], 1.0)

            h_t = [pp.tile([128, D], f32, tag=f"h{i}", name=f"h{i}") for i in range(TCH)]
            hT_t = [pp.tile([128, T], bf16, tag=f"hT{c}", name=f"hT{c}") for c in range(DCH)]
            qaug = [[pp.tile([128, L], bf16, tag=f"qa{s}_{hd}", name=f"qa{s}_{hd}") for hd in range(H)]
                    for s in range(SPC)]
            kaug = [[pp.tile([128, L], bf16, tag=f"ka{s}_{hd}", name=f"ka{s}_{hd}") for hd in range(H)]
                    for s in range(SPC)]
            v_t8 = [pp.tile([128, 2, H, 80], e4, tag=f"v8_{i}", name=f"v8_{i}")
                    for i in range(TCH // 2)]
            # ctxT aliases the hT tiles: within a layer, all hT reads (qk/v
            # matmuls) complete before attention writes ctx into the same
            # slots; Tile's WAR tracking enforces the ordering.
            ctxT = hT_t
            gel = [pp.tile([128, T], bf16, tag=f"g{f}", name=f"g{f}") for f in range(FCH)]

            # one-time init: zero the aug tiles, set ones/mask rows + v ones col
            for s in range(SPC):
                for hd in range(H):
                    nc.vector.memset(qaug[s][hd][:], 0.0)
                    nc.vector.memset(kaug[s][hd][:], 0.0)
                    nc.vector.memset(qaug[s][hd][DK:DK + 1, :], 1.0)
                    nc.sync.dma_start(kaug[s][hd][DK:DK + 1, :], mask_d[s:s + 1, :])
            for i in range(TCH // 2):
                nc.vector.memset(v_t8[i][:, :, :, DK:DK + 1], 8.0)
            for i in range(TCH):
                nc.sync.dma_start(h_t[i][:], h0_d[i * 128:(i + 1) * 128, :])

            def transpose_h():
                for c in range(DCH):
                    pst = psp.tile([128, T], f32, tag="ps2", bufs=3)
                    for i in range(TCH):
                        nc.tensor.transpose(
                            pst[:, i * 128:(i + 1) * 128],
                            h_t[i][:, c * 128:(c + 1) * 128], ident[:])
                    nc.vector.tensor_copy(hT_t[c][:], pst[:])

            def layernorm(gb):
                for i in range(TCH):
                    st = wk.tile([128, 2, 6], f32, tag="bnst", bufs=1)
                    for g in range(2):
                        nc.vector.bn_stats(st[:, g, :], h_t[i][:, g * 384:(g + 1) * 384])
                    mv = wk.tile([128, 2], f32, tag="bnmv", bufs=1)
                    nc.vector.bn_aggr(mv[:], st[:])
                    rstd = wk.tile([128, 1], f32, tag="rstd", bufs=1)
                    nc.scalar.activation(rstd[:], mv[:, 1:2], AF.Sqrt, bias=eps_t[:])
                    nc.vector.reciprocal_approx_fast(rstd[:], rstd[:])
                    nc.vector.tensor_scalar(h_t[i][:], h_t[i][:], mv[:, 0:1], rstd[:],
                                            op0=OP.subtract, op1=OP.mult)
                    if gb is not None:
                        nc.vector.tensor_tensor(h_t[i][:], h_t[i][:], gb[0][:],
                                                op=OP.mult)
                        nc.vector.tensor_tensor(h_t[i][:], h_t[i][:], gb[1][:],
                                                op=OP.add)

            transpose_h()

            for l in range(nl):
                if use_bias:
                    bqk_sb = wk.tile([128, 2 * DCH], f32, tag="bqk")
                    nc.sync.dma_start(bqk_sb[:],
                                      bqk_d[l].rearrange("b (c p) -> p (b c)", p=128))
                    b1_sb = wk.tile([128, FCH], f32, tag="b1sb")
                    nc.sync.dma_start(b1_sb[:],
                                      b1_d[l].rearrange("(c p) -> p c", p=128))
                    brow_sb = wk.tile([3, D], bf16, tag="brow")
                    nc.sync.dma_start(brow_sb[:], brow_d[l])
                if use_affine:
                    g_bc = wk.tile([128, D], f32, tag="gbc")
                    b_bc = wk.tile([128, D], f32, tag="bbc")
                    nc.gpsimd.dma_start(g_bc[:], lng_d[l:l + 1, :].to_broadcast((128, D)))
                    nc.gpsimd.dma_start(b_bc[:], lnb_d[l:l + 1, :].to_broadcast((128, D)))
                    gb = (g_bc, b_bc)
                else:
                    gb = None

                # ---- q^T / k^T (into aug tiles, scale q by 1/sqrt(dk)) ----
                for mat_d, aug, scale, bcol in ((wq_d, qaug, 1.0, 0),
                                                (wk_d, kaug, 1.0, 1)):
                    for dc in range(DCH):
                        wc = wk.tile([128, DCH, 128], bf16, tag="wqkcol", bufs=3, name="wqkc")
                        nc.sync.dma_start(
                            wc[:], mat_d[l][:, dc * 128:(dc + 1) * 128]
                            .rearrange("(c p) n -> p c n", p=128))
                        ps = psp.tile([128, T], f32, tag="ps2", bufs=3)
                        for c in range(DCH):
                            for s in range(SPC):
                                nc.tensor.matmul(ps[:, s * L:(s + 1) * L], wc[:, c, :],
                                                 hT_t[c][:, s * L:(s + 1) * L],
                                                 start=(c == 0), stop=(c == DCH - 1))
                        for s in range(SPC):
                            for hh in range(2):
                                src = ps[hh * 64:(hh + 1) * 64, s * L:(s + 1) * L]
                                dst = aug[s][2 * dc + hh][0:DK, :]
                                if use_bias:
                                    nc.scalar.activation(
                                        dst, src, AF.Identity, scale=scale,
                                        bias=bqk_sb[hh * 64:(hh + 1) * 64,
                                                    bcol * DCH + dc:bcol * DCH + dc + 1])
                                else:
                                    nc.vector.tensor_copy(dst, src)

                # ---- v (non-transposed, interleaved with ones cols) ----
                wv_rows = []
                for c in range(DCH):
                    wr = wp.tile([128, D], bf16, tag=f"wv{c}", name=f"wv{c}")
                    nc.sync.dma_start(wr[:], wv_d[l, c * 128:(c + 1) * 128, :])
                    wv_rows.append(wr)
                for i in range(TCH):
                    ps = psp.tile([128, D], f32, tag="ps2", bufs=3)
                    for c in range(DCH):
                        for off, wdt in ((0, 512), (512, 256)):
                            nc.tensor.matmul(ps[:, off:off + wdt],
                                             hT_t[c][:, i * 128:(i + 1) * 128],
                                             wv_rows[c][:, off:off + wdt],
                                             start=(c == 0),
                                             stop=(c == DCH - 1 and not use_bias))
                    if use_bias:
                        for off, wdt in ((0, 512), (512, 256)):
                            nc.tensor.matmul(ps[:, off:off + wdt], ones_row[:],
                                             brow_sb[0:1, off:off + wdt],
                                             start=False, stop=True)
                    nc.vector.tensor_scalar_mul(
                        v_t8[i // 2][:, i % 2, :, 0:DK],
                        ps[:].rearrange("p (h e) -> p h e", e=DK), 8.0)

                # ---- attention per (sequence, head) ----
                for s in range(SPC):
                    for hd in range(H):
                        nm = []
                        for tp_ in range(2):
                            ps = psp.tile([128, T], f32, tag="ps2", bufs=3)
                            for half in range(2):
                                tk = 2 * tp_ + half
                                nc.tensor.matmul(ps[:, half * L:(half + 1) * L],
                                                 kaug[s][hd][:, tk * 128:(tk + 1) * 128],
                                                 qaug[s][hd][:], start=True, stop=True)
                            t_nm = wk.tile([128, 2, L], e5, tag="numer", bufs=4,
                                           name="nm")
                            for half in range(2):
                                nc.scalar.activation(
                                    t_nm[:, half, :],
                                    ps[:, half * L:(half + 1) * L],
                                    AF.Exp, bias=shift_t[:])
                            nm.append(t_nm)
                        cps = psp.tile([128, L], f32, tag="ps1", bufs=2)
                        for tp_ in range(2):
                            nc.tensor.matmul(cps[0:DK + 1, :],
                                             v_t8[s * 2 + tp_][:, :, hd, 0:DK + 1],
                                             nm[tp_][:],
                                             start=(tp_ == 0), stop=(tp_ == 1),
                                             perf_mode=DR)
                        rec = wk.tile([1, L], f32, tag="rec", bufs=2)
                        nc.vector.tensor_copy(rec[:], cps[DK:DK + 1, :])
                        nc.vector.reciprocal_approx_fast(rec[:], rec[:])
                        recb = wk.tile([DK, L], f32, tag="recb", bufs=2)
                        nc.gpsimd.partition_broadcast(recb[:], rec[:])
                        dst = ctxT[hd // 2][(hd % 2) * 64:(hd % 2) * 64 + 64,
                                            s * L:(s + 1) * L]
                        nc.vector.tensor_tensor(dst, cps[0:DK, :], recb[:],
                                                op=OP.mult)

                # ---- attn output + residual ----
                wo_rows = []
                for c in range(DCH):
                    wr = wp.tile([128, D], bf16, tag=f"wo{c}", name=f"wo{c}")
                    nc.sync.dma_start(wr[:], wo_d[l, c * 128:(c + 1) * 128, :])
                    wo_rows.append(wr)
                for i in range(TCH):
                    ps = psp.tile([128, D], f32, tag="ps2", bufs=3)
                    for c in range(DCH):
                        for off, wdt in ((0, 512), (512, 256)):
                            nc.tensor.matmul(ps[:, off:off + wdt],
                                             ctxT[c][:, i * 128:(i + 1) * 128],
                                             wo_rows[c][:, off:off + wdt],
                                             start=(c == 0),
                                             stop=(c == DCH - 1 and not use_bias))
                    if use_bias:
                        for off, wdt in ((0, 512), (512, 256)):
                            nc.tensor.matmul(ps[:, off:off + wdt], ones_row[:],
                                             brow_sb[1:2, off:off + wdt],
                                             start=False, stop=True)
                    nc.vector.tensor_tensor(h_t[i][:], ps[:], h_t[i][:], op=OP.add)

                layernorm(gb)
                transpose_h()

                # ---- ffn1 + gelu (transposed output) ----
                for f in range(FCH):
                    w1c = wk.tile([128, DCH, 128], bf16, tag="w1col", bufs=3, name="w1c")
                    nc.sync.dma_start(
                        w1c[:], w1_d[l][:, f * 128:(f + 1) * 128]
                        .rearrange("(c p) n -> p c n", p=128))
                    ps = psp.tile([128, T], f32, tag="ps2", bufs=3)
                    for c in range(DCH):
                        for g in range(2):
                            nc.tensor.matmul(ps[:, g * 512:(g + 1) * 512], w1c[:, c, :],
                                             hT_t[c][:, g * 512:(g + 1) * 512],
                                             start=(c == 0), stop=(c == DCH - 1))
                    nc.scalar.activation(
                        gel[f][:], ps[:], AF.Gelu,
                        bias=(b1_sb[:, f:f + 1] if use_bias else 0.0))

                # ---- ffn2 + residual ----
                for dh in range(2):
                    w2_rows = []
                    for f in range(FCH):
                        wr = wp.tile([128, DH], bf16, tag=f"w2r{f}", name=f"w2r{f}")
                        nc.sync.dma_start(
                            wr[:], w2_d[l, f * 128:(f + 1) * 128,
                                        dh * DH:(dh + 1) * DH])
                        w2_rows.append(wr)
                    for i in range(TCH):
                        ps = psp.tile([128, DH], f32, tag="ps1", bufs=2)
                        for f in range(FCH):
                            nc.tensor.matmul(ps[:], gel[f][:, i * 128:(i + 1) * 128],
                                             w2_rows[f][:],
                                             start=(f == 0),
                                             stop=(f == FCH - 1 and not use_bias))
                        if use_bias:
                            nc.tensor.matmul(ps[:], ones_row[:],
                                             brow_sb[2:3, dh * DH:(dh + 1) * DH],
                                             start=False, stop=True)
                        nc.vector.tensor_tensor(h_t[i][:, dh * DH:(dh + 1) * DH], ps[:],
                                                h_t[i][:, dh * DH:(dh + 1) * DH],
                                                op=OP.add)

                layernorm(gb)
                if l < nl - 1:
                    transpose_h()

            for i in range(TCH):
                nc.sync.dma_start(out_d[i * 128:(i + 1) * 128, :], h_t[i][:])

    nc.compile()
    return nc, names


def _get_program(nl, use_bias, use_affine):
    key = (nl, use_bias, use_affine)
    if key not in _CACHE:
        _CACHE[key] = _build_program(nl, use_bias, use_affine)
    return _CACHE[key]


def kernel(**inputs):
    from concourse import bass_utils

    x = np.asarray(inputs["x"])
    tok = np.asarray(inputs["token_emb"], np.float32)
    pe = np.asarray(inputs["pe"], np.float32)
    to_bf = lambda a: np.asarray(a, np.float32).astype(ml_dtypes.bfloat16)

    h0 = tok[x] + pe[None]                                   # (B, L, D) f32
    maskrow = np.where(x > 0, 0.0, -1e9).astype(np.float32)  # (B, L)

    nl = int(os.environ.get("BERT_NL", str(NL)))
    bias_arrs = [np.asarray(inputs[k], np.float32)[:nl]
                 for k in ("bq", "bk", "bv", "bo", "b1", "b2")]
    use_bias = any(np.any(a != 0.0) for a in bias_arrs)
    lng = np.asarray(inputs["ln_g"], np.float32)[:nl]
    lnb = np.asarray(inputs["ln_b"], np.float32)[:nl]
    use_affine = bool(np.any(lng != 1.0) or np.any(lnb != 0.0))

    nc, names = _get_program(nl, use_bias, use_affine)

    shared = {
        "wq": to_bf(np.asarray(inputs["Wq"][:nl], np.float32) * 0.125),
        "wk": to_bf(inputs["Wk"][:nl]),
        "wv": to_bf(inputs["Wv"][:nl]), "wo": to_bf(inputs["Wo"][:nl]),
        "w1": to_bf(inputs["W1"][:nl]), "w2": to_bf(inputs["W2"][:nl]),
    }
    if use_bias:
        bq, bk, bv, bo, b1, b2 = bias_arrs
        shared["bqk"] = np.stack([bq, bk], axis=1).astype(np.float32)
        shared["b1"] = b1.astype(np.float32)
        shared["brow"] = to_bf(np.stack([bv, bo, b2], axis=1))
    if use_affine:
        shared["lng"] = lng
        shared["lnb"] = lnb

    in_maps = []
    for c in range(NCORES):
        im = dict(shared)
        im["h0"] = np.ascontiguousarray(
            h0[SPC * c:SPC * (c + 1)].reshape(T, D), dtype=np.float32)
        im["maskrow"] = to_bf(maskrow[SPC * c:SPC * (c + 1)])
        in_maps.append(im)

    trace = os.environ.get("BERT_TRACE", "0") == "1"
    res = bass_utils.run_bass_kernel_spmd(
        nc, in_maps, core_ids=list(range(NCORES)), trace=trace)
    if trace:
        print(f"HW exec time: {res.exec_time_ns} ns")

    out = np.stack([np.asarray(res.results[c]["out"]).reshape(SPC, L, D)
                    for c in range(NCORES)])
    return out.reshape(B, L, D).astype(np.float32)

